# revision 1
# baseline (speedup 1.0000x reference)
"""Causal self-attention TRN2 Bass kernel (B=4, T=2048, C=1024, H=16, D=64, fp32).

Sharding: 8 cores = 4 batches x 2 head-groups (8 heads each). Each core computes
its batch's QKV for its heads, causal flash-style attention, and a partial
output projection; the host sums the two head-group partials per batch.

Device dataflow (all matmuls in float32r — ~1.5e-4 rel, 4x fp32 speed):
  phase 1: qkT = [Wq|Wk]^T x  (via lhsT=W blocks, rhs=x^T chunks) -> DRAM scratch
           v   = x Wv         (via lhsT=x^T blocks, rhs=Wv)       -> DRAM scratch
  phase 2: per head-pair: S^T[k,q] = K^T.T Q^T (row-tiled 2 heads in PE),
           causal mask only on diagonal 128x128 blocks, exp on ACT (scale=1/8),
           O^T/sums fused: lhsT=[V|ones] so psum rows 0..63=O^T, 64..127=sums,
           divide via DVE reciprocal+mult -> o^T in SBUF.
  phase 3: y^T = W_proj^T o^T (partial over this core's heads) -> DRAM out.
Host: y[b] = (yT[2b] + yT[2b+1]).T
"""

import numpy as np
from contextlib import ExitStack

import concourse.bass as bass
import concourse.tile as tile
from concourse import bacc, mybir
from concourse.bass import ts
from concourse.bass_utils import run_bass_kernel_spmd

N_CORES = 8
B, T, C, H, D = 4, 2048, 1024, 16, 64
CB = C // 128          # 8 contraction blocks
NKB = T // 128         # 16 key blocks
NQC = T // 512         # 4 query chunks
NEG = -1.0e9

F32 = mybir.dt.float32
F32R = mybir.dt.float32r
AF = mybir.ActivationFunctionType
OP = mybir.AluOpType

_CACHE = {}


def _build(phases=(1, 2, 3), reps=1):
    nc = bacc.Bacc("TRN2", target_bir_lowering=False, debug=False, num_devices=N_CORES)

    xT = nc.dram_tensor("xT", [C, T], F32R, kind="ExternalInput").ap()
    w_qk = nc.dram_tensor("w_qk", [C, 1024], F32R, kind="ExternalInput").ap()
    w_v = nc.dram_tensor("w_v", [C, 512], F32R, kind="ExternalInput").ap()
    w_pr = nc.dram_tensor("w_pr", [512, C], F32R, kind="ExternalInput").ap()
    b_qk = nc.dram_tensor("b_qk", [1024], F32, kind="ExternalInput").ap()
    b_v = nc.dram_tensor("b_v", [128, 512], F32, kind="ExternalInput").ap()
    b_pr = nc.dram_tensor("b_pr", [C], F32, kind="ExternalInput").ap()
    yT = nc.dram_tensor("yT", [C, T], F32, kind="ExternalOutput").ap()

    xT_r = xT.rearrange("(cb p) t -> p cb t", p=128)
    w_qk_r = w_qk.rearrange("(cb p) m -> p cb m", p=128)
    w_v_r = w_v.rearrange("(cb p) m -> p cb m", p=128)
    w_pr_r = w_pr.rearrange("(pb p) m -> p pb m", p=128)
    b_qk_r = b_qk.rearrange("(m p) -> p m", p=128)
    b_pr_r = b_pr.rearrange("(m p) -> p m", p=128)

    with tile.TileContext(nc) as tc:
        with ExitStack() as ctx:
            # pools
            io = ctx.enter_context(tc.tile_pool(name="io", bufs=2))        # 16KB tiles
            wqk_p = ctx.enter_context(tc.tile_pool(name="wqk", bufs=1))
            w2_p = ctx.enter_context(tc.tile_pool(name="w2", bufs=1))
            wpr_p = ctx.enter_context(tc.tile_pool(name="wpr", bufs=1))
            qkt_p = ctx.enter_context(tc.tile_pool(name="qkt", bufs=4))
            stage_p = ctx.enter_context(tc.tile_pool(name="stage", bufs=4))
            p_p = ctx.enter_context(tc.tile_pool(name="pp", bufs=6))
            ot_p = ctx.enter_context(tc.tile_pool(name="ot", bufs=1))
            misc = ctx.enter_context(tc.tile_pool(name="misc", bufs=1))
            rec_p = ctx.enter_context(tc.tile_pool(name="rec", bufs=4))
            dram = ctx.enter_context(tc.tile_pool(name="dram", bufs=1, space="DRAM"))
            ps_qkv = ctx.enter_context(tc.tile_pool(name="ps_qkv", bufs=2, space="PSUM"))
            ps_s_p = ctx.enter_context(tc.tile_pool(name="ps_s", bufs=3, space="PSUM"))
            ps_o_p = ps_qkv  # share the 2 slots; phases are disjoint in time

            # constants
            b_qk_sb = misc.tile([128, 8], F32)
            nc.sync.dma_start(b_qk_sb[:], b_qk_r)
            b_v_sb = misc.tile([128, 512], F32)
            nc.sync.dma_start(b_v_sb[:], b_v)
            b_pr_sb = misc.tile([128, 8], F32)
            nc.sync.dma_start(b_pr_sb[:], b_pr_r)
            ones_sb = misc.tile([128, 64], F32)
            nc.gpsimd.memset(ones_sb[:], 1.0)
            tri = misc.tile([128, 128], F32)
            nc.gpsimd.memset(tri[:], 0.0)
            # 0 where q(free) >= k(partition), NEG where q < k
            nc.gpsimd.affine_select(
                out=tri[:], in_=tri[:], compare_op=OP.is_ge, fill=NEG,
                base=0, pattern=[[1, 128]], channel_multiplier=-1,
            )


            # weights
            w_qk_sb = wqk_p.tile([128, CB, 1024], F32R)
            nc.sync.dma_start(w_qk_sb[:], w_qk_r)
            w_v_sb = w2_p.tile([128, CB, 512], F32R, tag="w16k")
            nc.sync.dma_start(w_v_sb[:], w_v_r)
            w_pr_sb = wpr_p.tile([128, 4, 1024], F32R, name="w_pr_sb")
            nc.sync.dma_start(w_pr_sb[:], w_pr_r)

            for _rep in range(reps):
              # DRAM scratch — split per block for tight RAW deps (earlier prefetch)
              qkT_d = [dram.tile([128, T], F32R, name=f"qkT_d{_rep}_{m}", tag=f"qkT_d{m}")
                       for m in range(8)]
              v_d = [dram.tile([4, 128, 512], F32R, name=f"v_d{_rep}_{t}", tag=f"v_d{t}")
                     for t in range(4)]
              # ---------- phase 1: QKV ----------
              for tch in range(4 if 1 in phases else 0):
                  x_t = io.tile([128, CB, 512], F32R, tag="io16k", name=f"x_{tch}")
                  nc.sync.dma_start(x_t[:], xT_r[:, :, ts(tch, 512)])
                  for mp in range(4):
                      ps = ps_s_p.tile([128, 2, 512], F32, tag="ps_s", name=f"qk_{tch}_{mp}")
                      for cb in range(CB):
                          for h in (0, 1):
                              nc.tensor.matmul(
                                  ps[:, h], w_qk_sb[:, cb, ts(2 * mp + h, 128)], x_t[:, cb],
                                  start=(cb == 0), stop=(cb == CB - 1),
                              )
                      for h in (0, 1):
                          m = 2 * mp + h
                          st = stage_p.tile([128, 512], F32R, tag="stage", name=f"qks_{tch}_{m}")
                          nc.scalar.activation(st[:], ps[:, h], AF.Identity, bias=b_qk_sb[:, m : m + 1])
                          nc.sync.dma_start(qkT_d[m][:, ts(tch, 512)], st[:])
                  for vp in range(2):
                      ps = ps_s_p.tile([128, 2, 512], F32, tag="ps_s", name=f"v_{tch}_{vp}")
                      for cb in range(CB):
                          for h in (0, 1):
                              nc.tensor.matmul(
                                  ps[:, h], x_t[:, cb, ts(2 * vp + h, 128)], w_v_sb[:, cb],
                                  start=(cb == 0), stop=(cb == CB - 1),
                              )
                      for h in (0, 1):
                          tb = tch * 4 + 2 * vp + h
                          st = stage_p.tile([128, 512], F32R, tag="stage", name=f"vs_{tb}")
                          nc.vector.tensor_tensor(st[:], ps[:, h], b_v_sb[:], OP.add)
                          nc.sync.dma_start(v_d[tb // 4][tb % 4], st[:])

              # ---------- phase 2: attention, one head-pair at a time ----------
              oT = ot_p.tile([128, 4, T], F32R, name="oT")
              if 2.5 in phases:
                  nc.vector.tensor_copy(
                      oT[:], ones_sb[:, 0:1, None].to_broadcast((128, 4, T)))
              for pr in range(4 if 2 in phases else 0):
                  qT = qkt_p.tile([128, T], F32R, tag="qkt", name=f"qT_{pr}")
                  kT = qkt_p.tile([128, T], F32R, tag="qkt", name=f"kT_{pr}")
                  for tch in range(4):
                      nc.sync.dma_start(qT[:, ts(tch, 512)], qkT_d[pr][:, ts(tch, 512)])
                      nc.sync.dma_start(kT[:, ts(tch, 512)], qkT_d[4 + pr][:, ts(tch, 512)])
                  v_aug = io.tile([128, NKB, 2, 128], F32R, tag="io16k", name=f"va_{pr}")
                  for tg in range(4):
                      for j in (0, 1):
                          nc.sync.dma_start(
                              v_aug[:, 4 * tg : 4 * tg + 4, j, 0:64],
                              v_d[tg][:, :, pr * 128 + j * 64 : pr * 128 + (j + 1) * 64]
                              .rearrange("kb p d -> p kb d"),
                          )
                  nc.vector.tensor_copy(
                      v_aug[:, :, :, 64:128],
                      ones_sb[:, None, None, :].to_broadcast((128, NKB, 2, 64)),
                  )

                  for qc in range(NQC):
                      nkb = 4 * qc + 4
                      ps_o = [
                          ps_o_p.tile([128, 512], F32, tag="ps_qkv", name=f"o_{pr}_{qc}_{j}")
                          for j in (0, 1)
                      ]

                      # software-pipelined: issue S(kb+1) before AV(kb)
                      ps_s = [None] * nkb
                      p_ts = [None] * nkb

                      def s_step(kb):
                          r = kb - 4 * qc
                          qlo = 128 * r if r > 0 else 0
                          s = ps_s_p.tile([128, 2, 512], F32, tag="ps_s",
                                          name=f"s_{pr}_{qc}_{kb}")
                          for j in ((0,) if 2.9 in phases else (0, 1)):
                              pb = j * 64
                              nc.tensor.matmul(
                                  s[:, j, qlo:512],
                                  kT[pb : pb + 64, ts(kb, 128)],
                                  qT[pb : pb + 64, qc * 512 + qlo : (qc + 1) * 512],
                                  start=True, stop=True, tile_position=(pb, 0),
                              )
                          if 2.9 in phases:
                              nc.vector.tensor_copy(s[:, 1], s[:, 0])
                          if r >= 0:
                              nc.vector.tensor_tensor(
                                  s[:, :, qlo : qlo + 128], s[:, :, qlo : qlo + 128],
                                  tri[:, None, :].to_broadcast((128, 2, 128)), OP.add,
                              )
                          ps_s[kb] = s

                      def av_step(kb):
                          r = kb - 4 * qc
                          qlo = 128 * r if r > 0 else 0
                          p_t = p_p.tile([128, 2, 512], F32R, tag="p",
                                         name=f"p_{pr}_{qc}_{kb}")
                          nc.scalar.activation(
                              p_t[:, :, qlo:512], ps_s[kb][:, :, qlo:512],
                              AF.Exp, scale=0.125,
                          )
                          for j in (0, 1):
                              nc.tensor.matmul(
                                  ps_o[j][:, qlo:512], v_aug[:, kb, j], p_t[:, j, qlo:512],
                                  start=(kb == 0), stop=(kb == nkb - 1),
                              )

                      s_step(0)
                      for kb in range(nkb):
                          if kb + 1 < nkb:
                              s_step(kb + 1)
                          av_step(kb)

                      for j in (0, 1):
                          rec = rec_p.tile([64, 512], F32, tag="rec",
                                           name=f"rec_{pr}_{qc}_{j}")
                          nc.vector.reciprocal(rec[:], ps_o[j][64:128, :])
                          nc.vector.tensor_tensor(
                              oT[j * 64 : (j + 1) * 64, pr, ts(qc, 512)],
                              ps_o[j][0:64, :], rec[:], OP.mult,
                          )

              # ---------- phase 3: projection ----------
              if 3 not in phases:
                  st0 = stage_p.tile([128, 512], F32, tag="stage", name="dummy_out")
                  nc.gpsimd.memset(st0[:], 0.0)
                  nc.sync.dma_start(yT.rearrange("(m p) t -> p m t", p=128)[:, 0, 0:512], st0[:])
              for m in range(8 if 3 in phases else 0):
                  for tch in range(4):
                      ps = ps_qkv.tile([128, 512], F32, tag="ps_qkv", name=f"y_{m}_{tch}")
                      for pb in range(4):
                          nc.tensor.matmul(
                              ps[:], w_pr_sb[:, pb, ts(m, 128)], oT[:, pb, ts(tch, 512)],
                              start=(pb == 0), stop=(pb == 3),
                          )
                      st = stage_p.tile([128, 512], F32, tag="stage", name=f"ys_{m}_{tch}")
                      nc.scalar.activation(st[:], ps[:], AF.Identity, bias=b_pr_sb[:, m : m + 1])
                      nc.sync.dma_start(yT.rearrange("(m p) t -> p m t", p=128)[:, m, ts(tch, 512)], st[:])

    nc.compile()
    return nc


def _in_maps(x, W_attn, b_attn, W_proj, b_proj):
    maps = []
    for b in range(B):
        for g in range(2):
            cs = slice(g * 512, (g + 1) * 512)
            maps.append({
                "xT": np.ascontiguousarray(x[b].T),
                "w_qk": np.ascontiguousarray(
                    np.concatenate([W_attn[:, cs], W_attn[:, 1024 + cs.start : 1024 + cs.stop]], axis=1)),
                "w_v": np.ascontiguousarray(W_attn[:, 2048 + cs.start : 2048 + cs.stop]),
                "w_pr": np.ascontiguousarray(W_proj[cs, :]),
                "b_qk": np.ascontiguousarray(
                    np.concatenate([b_attn[cs], b_attn[1024 + cs.start : 1024 + cs.stop]])),
                "b_v": np.ascontiguousarray(
                    np.tile(b_attn[2048 + cs.start : 2048 + cs.stop][None, :], (128, 1))),
                "b_pr": np.ascontiguousarray(b_proj),
            })
    return maps


def kernel(x, W_attn, b_attn, W_proj, b_proj):
    x = np.asarray(x, dtype=np.float32)
    W_attn = np.asarray(W_attn, dtype=np.float32)
    b_attn = np.asarray(b_attn, dtype=np.float32)
    W_proj = np.asarray(W_proj, dtype=np.float32)
    b_proj = np.asarray(b_proj, dtype=np.float32)

    if "nc" not in _CACHE:
        _CACHE["nc"] = _build()
    nc = _CACHE["nc"]

    maps = _in_maps(x, W_attn, b_attn, W_proj, b_proj)
    last_exc = None
    for attempt in range(3):
        try:
            res = run_bass_kernel_spmd(nc, maps, core_ids=list(range(N_CORES)))
            break
        except Exception as exc:  # transient device wedges recover on retry
            last_exc = exc
            if attempt == 2:
                raise
            import time as _time
            _time.sleep(5)
    y = np.empty((B, T, C), dtype=np.float32)
    for b in range(B):
        y[b] = (res.results[2 * b]["yT"] + res.results[2 * b + 1]["yT"]).T
    return y



# revision 6
# speedup vs baseline: 1.5278x; 1.5278x over previous
"""Causal self-attention TRN2 Bass kernel (B=4, T=2048, C=1024, H=16, D=64, fp32).

Sharding: 8 cores = 4 batches x 2 head-groups (8 heads each). Each core computes
its batch's QKV for its heads, causal flash-style attention, and a partial
output projection; the host sums the two head-group partials per batch.

v2: fully SBUF-resident, fused qc-outer pipeline.
  Per 512-query chunk tch (=qc):
    QKV: q_t[pr], k_sb[:, pr, tch], va[kb] computed from streamed x chunk
         (PE matmuls f32r; Pool drains PSUM->SBUF with bias add)
    attention qc=tch for all 4 head-pairs pr (needs only keys <= chunk end):
         S^T[k,q] strips (f32r, diagonal strips padded to >=256 rows),
         causal mask via DVE add of NEG triangle, exp on ACT (scale=1/8,
         bf16 out), AV with [V|ones] stationary (bf16) accumulating O^T and
         softmax sums in one PSUM tile; Pool drains, DVE reciprocal,
         DVE/Pool multiply -> oT (bf16)
    proj(tch): y^T partial = W_proj^T oT (bf16 x bf16), Pool bias-drain,
         DMA out.
Host: y[b] = (yT[2b] + yT[2b+1]).T
"""

import numpy as np
from contextlib import ExitStack

import concourse.bass as bass
import concourse.tile as tile
from concourse import bacc, mybir
from concourse.bass import ts
from concourse.bass_utils import run_bass_kernel_spmd

N_CORES = 8
B, T, C, H, D = 4, 2048, 1024, 16, 64
CB = C // 128          # 8 contraction blocks
NEG = -1.0e9

F32 = mybir.dt.float32
F32R = mybir.dt.float32r
BF16 = mybir.dt.bfloat16
AF = mybir.ActivationFunctionType
OP = mybir.AluOpType

_CACHE = {}

# query-strip low offset by diagonal position r (r = kb - 4*qc; r<0 off-diag)
_QLO = {0: 0, 1: 128, 2: 256, 3: 256}


def _build(phases=(1, 2, 3), reps=1):
    nc = bacc.Bacc("TRN2", target_bir_lowering=False, debug=False, num_devices=N_CORES)

    xT = nc.dram_tensor("xT", [C, T], F32R, kind="ExternalInput").ap()
    w_qk = nc.dram_tensor("w_qk", [C, 1024], F32R, kind="ExternalInput").ap()
    w_v = nc.dram_tensor("w_v", [C, 512], F32R, kind="ExternalInput").ap()
    w_pr = nc.dram_tensor("w_pr", [512, C], BF16, kind="ExternalInput").ap()
    b_qk = nc.dram_tensor("b_qk", [1024], F32, kind="ExternalInput").ap()
    b_v = nc.dram_tensor("b_v", [128, 512], F32, kind="ExternalInput").ap()
    b_pr = nc.dram_tensor("b_pr", [C], F32, kind="ExternalInput").ap()
    yT = nc.dram_tensor("yT", [C, T], F32, kind="ExternalOutput").ap()

    xT_r = xT.rearrange("(cb p) t -> p cb t", p=128)
    w_qk_r = w_qk.rearrange("(cb p) m -> p cb m", p=128)
    w_v_r = w_v.rearrange("(cb p) m -> p cb m", p=128)
    w_pr_r = w_pr.rearrange("(pb p) m -> p pb m", p=128)
    b_qk_r = b_qk.rearrange("(m p) -> p m", p=128)
    b_pr_r = b_pr.rearrange("(m p) -> p m", p=128)
    yT_r = yT.rearrange("(m p) t -> p m t", p=128)

    with tile.TileContext(nc) as tc:
        with ExitStack() as ctx:
            wqk_p = ctx.enter_context(tc.tile_pool(name="wqk", bufs=1))
            w2_p = ctx.enter_context(tc.tile_pool(name="w2", bufs=1))
            wpr_p = ctx.enter_context(tc.tile_pool(name="wpr", bufs=1))
            k_pl = ctx.enter_context(tc.tile_pool(name="kp", bufs=1))
            va_pl = ctx.enter_context(tc.tile_pool(name="vap", bufs=1))
            ot_pl = ctx.enter_context(tc.tile_pool(name="otp", bufs=1))
            x_pl = ctx.enter_context(tc.tile_pool(name="xp", bufs=2))
            q_pl = ctx.enter_context(tc.tile_pool(name="qp", bufs=2))
            p_pl = ctx.enter_context(tc.tile_pool(name="pp", bufs=4))
            ys_pl = ctx.enter_context(tc.tile_pool(name="ysp", bufs=2))
            rc_pl = ctx.enter_context(tc.tile_pool(name="rcp", bufs=1))
            misc = ctx.enter_context(tc.tile_pool(name="misc", bufs=1))
            ps_s = ctx.enter_context(tc.tile_pool(name="ps_s", bufs=3, space="PSUM"))
            ps_o = ctx.enter_context(tc.tile_pool(name="ps_o", bufs=1, space="PSUM"))

            # constants
            b_qk_sb = misc.tile([128, 8], F32)
            nc.sync.dma_start(b_qk_sb[:], b_qk_r)
            b_v_sb = misc.tile([128, 512], F32)
            nc.sync.dma_start(b_v_sb[:], b_v)
            b_pr_sb = misc.tile([128, 8], F32)
            nc.sync.dma_start(b_pr_sb[:], b_pr_r)
            # tri2: [128k, 256q]: cols 0:128 all NEG, cols 128:256 NEG where q<k
            tri2 = misc.tile([128, 256], F32)
            nc.gpsimd.memset(tri2[:], 0.0)
            nc.gpsimd.affine_select(
                out=tri2[:], in_=tri2[:], compare_op=OP.is_ge, fill=NEG,
                base=-128, pattern=[[1, 256]], channel_multiplier=-1,
            )
            tri = tri2[:, 128:256]

            # weights
            w_qk_sb = wqk_p.tile([128, CB, 1024], F32R)
            nc.sync.dma_start(w_qk_sb[:], w_qk_r)
            w_v_sb = w2_p.tile([128, CB, 512], F32R)
            nc.sync.dma_start(w_v_sb[:], w_v_r)
            w_pr_sb = wpr_p.tile([128, 4, 1024], BF16)
            nc.sync.dma_start(w_pr_sb[:], w_pr_r)

            # persistent activations (shared across reps; rewritten per rep)
            k_sb = k_pl.tile([128, 4, T], F32R, name="k_sb")
            va = va_pl.tile([128, 16, 8, 2, 64], BF16, name="va")
            oT = ot_pl.tile([128, 4, T], BF16, name="oT")
            nc.gpsimd.memset(va[:, :, :, 1, :], 1.0)

            for _rep in range(reps):
                for tch in range(4):
                    # ---- QKV for this 512-t chunk ----
                    x_t = x_pl.tile([128, CB, 512], F32R, tag="x",
                                    name=f"x_{_rep}_{tch}")
                    nc.sync.dma_start(x_t[:], xT_r[:, :, ts(tch, 512)])
                    q_t = q_pl.tile([128, 4, 512], F32R, tag="q",
                                    name=f"q_{_rep}_{tch}")
                    for mp in range(4):
                        ps = ps_s.tile([128, 2, 512], F32, tag="ps_s",
                                       name=f"qk_{_rep}_{tch}_{mp}")
                        for cb in range(CB):
                            for h in (0, 1):
                                nc.tensor.matmul(
                                    ps[:, h], w_qk_sb[:, cb, ts(2 * mp + h, 128)],
                                    x_t[:, cb],
                                    start=(cb == 0), stop=(cb == CB - 1),
                                )
                        for h in (0, 1):
                            m = 2 * mp + h
                            if m < 4:
                                nc.vector.tensor_scalar_add(
                                    q_t[:, m, :], ps[:, h], b_qk_sb[:, m : m + 1])
                            else:
                                nc.vector.tensor_scalar_add(
                                    k_sb[:, m - 4, ts(tch, 512)], ps[:, h],
                                    b_qk_sb[:, m : m + 1])
                    for vp in range(2):
                        ps = ps_s.tile([128, 2, 512], F32, tag="ps_s",
                                       name=f"v_{_rep}_{tch}_{vp}")
                        for cb in range(CB):
                            for h in (0, 1):
                                nc.tensor.matmul(
                                    ps[:, h], x_t[:, cb, ts(2 * vp + h, 128)],
                                    w_v_sb[:, cb],
                                    start=(cb == 0), stop=(cb == CB - 1),
                                )
                        for h in (0, 1):
                            tb = tch * 4 + 2 * vp + h
                            nc.vector.tensor_tensor(
                                va[:, tb, :, 0, :],
                                ps[:, h].rearrange("p (s d) -> p s d", s=8),
                                b_v_sb.rearrange("p (s d) -> p s d", s=8),
                                OP.add)

                    # ---- attention qc = tch, all head-pairs ----
                    qc = tch
                    nkb = 4 * qc + 4
                    for pr in range(4):
                        po = ps_o.tile([128, 2, 512], F32, tag="ps_o",
                                       name=f"o_{_rep}_{qc}_{pr}")
                        s_tiles = {}

                        def s_step(kb, pr=pr, s_tiles=s_tiles):
                            r = kb - 4 * qc
                            qlo = _QLO.get(r, 0)
                            s = ps_s.tile([128, 2, 512], F32, tag="ps_s",
                                          name=f"s_{_rep}_{qc}_{pr}_{kb}")
                            for j in (0, 1):
                                pb = 64 * j
                                nc.tensor.matmul(
                                    s[:, j, qlo:512],
                                    k_sb[pb : pb + 64, pr, ts(kb, 128)],
                                    q_t[pb : pb + 64, pr, qlo:512],
                                    start=True, stop=True, tile_position=(pb, 0),
                                )
                            if 0 <= r <= 2:
                                c0 = 128 * r
                                nc.vector.tensor_tensor(
                                    s[:, :, c0 : c0 + 128], s[:, :, c0 : c0 + 128],
                                    tri[:, None, :].to_broadcast((128, 2, 128)),
                                    OP.add)
                            elif r == 3:
                                nc.vector.tensor_tensor(
                                    s[:, :, 256:512], s[:, :, 256:512],
                                    tri2[:, None, :].to_broadcast((128, 2, 256)),
                                    OP.add)
                            s_tiles[kb] = (s, qlo)

                        def av_step(kb, pr=pr, po=po, s_tiles=s_tiles):
                            s, qlo = s_tiles.pop(kb)
                            p_t = p_pl.tile([128, 2, 512], BF16, tag="p",
                                            name=f"p_{_rep}_{qc}_{pr}_{kb}")
                            nc.scalar.activation(
                                p_t[:, :, qlo:512], s[:, :, qlo:512],
                                AF.Exp, scale=0.125)
                            for j in (0, 1):
                                s0 = 2 * pr + j
                                nc.tensor.matmul(
                                    po[:, j, qlo:512],
                                    va[:, kb, s0].rearrange("p a b -> p (a b)"),
                                    p_t[:, j, qlo:512],
                                    start=(kb == 0), stop=(kb == nkb - 1),
                                )

                        s_step(0)
                        for kb in range(nkb):
                            if kb + 1 < nkb:
                                s_step(kb + 1)
                            av_step(kb)

                        # normalize straight from PSUM
                        rec = rc_pl.tile([64, 2, 512], F32, tag="rec",
                                         name=f"rec_{_rep}_{qc}_{pr}")
                        nc.vector.reciprocal(rec[:], po[64:128])
                        for j in (0, 1):
                            nc.vector.tensor_tensor(
                                oT[j * 64 : (j + 1) * 64, pr, ts(qc, 512)],
                                po[0:64, j], rec[:, j], OP.mult)

                    # ---- projection for this chunk ----
                    for mp in range(4):
                        ps = ps_s.tile([128, 2, 512], F32, tag="ps_s",
                                       name=f"y_{_rep}_{tch}_{mp}")
                        for h in (0, 1):
                            for pb in range(4):
                                nc.tensor.matmul(
                                    ps[:, h], w_pr_sb[:, pb, ts(2 * mp + h, 128)],
                                    oT[:, pb, ts(tch, 512)],
                                    start=(pb == 0), stop=(pb == 3),
                                )
                        yst = ys_pl.tile([128, 2, 512], F32, tag="ystg",
                                         name=f"yst_{_rep}_{tch}_{mp}")
                        for h in (0, 1):
                            nc.vector.tensor_scalar_add(
                                yst[:, h], ps[:, h],
                                b_pr_sb[:, 2 * mp + h : 2 * mp + h + 1])
                        nc.sync.dma_start(
                            yT_r[:, 2 * mp : 2 * mp + 2, ts(tch, 512)], yst[:])

    nc.compile()
    return nc


def _in_maps(x, W_attn, b_attn, W_proj, b_proj):
    maps = []
    for b in range(B):
        for g in range(2):
            cs = slice(g * 512, (g + 1) * 512)
            maps.append({
                "xT": np.ascontiguousarray(x[b].T),
                "w_qk": np.ascontiguousarray(
                    np.concatenate([W_attn[:, cs], W_attn[:, 1024 + cs.start : 1024 + cs.stop]], axis=1)),
                "w_v": np.ascontiguousarray(W_attn[:, 2048 + cs.start : 2048 + cs.stop]),
                "w_pr": np.ascontiguousarray(W_proj[cs, :]).astype(mybir.dt.np(BF16)),
                "b_qk": np.ascontiguousarray(
                    np.concatenate([b_attn[cs], b_attn[1024 + cs.start : 1024 + cs.stop]])),
                "b_v": np.ascontiguousarray(
                    np.tile(b_attn[2048 + cs.start : 2048 + cs.stop][None, :], (128, 1))),
                "b_pr": np.ascontiguousarray(b_proj),
            })
    return maps


def kernel(x, W_attn, b_attn, W_proj, b_proj):
    x = np.asarray(x, dtype=np.float32)
    W_attn = np.asarray(W_attn, dtype=np.float32)
    b_attn = np.asarray(b_attn, dtype=np.float32)
    W_proj = np.asarray(W_proj, dtype=np.float32)
    b_proj = np.asarray(b_proj, dtype=np.float32)

    if "nc" not in _CACHE:
        _CACHE["nc"] = _build()
    nc = _CACHE["nc"]

    maps = _in_maps(x, W_attn, b_attn, W_proj, b_proj)
    last_exc = None
    for attempt in range(3):
        try:
            res = run_bass_kernel_spmd(nc, maps, core_ids=list(range(N_CORES)))
            break
        except Exception as exc:  # transient device wedges recover on retry
            last_exc = exc
            if attempt == 2:
                raise
            import time as _time
            _time.sleep(5)
    y = np.empty((B, T, C), dtype=np.float32)
    for b in range(B):
        y[b] = (res.results[2 * b]["yT"] + res.results[2 * b + 1]["yT"]).T
    return y


# revision 8
# speedup vs baseline: 16.2721x; 10.6510x over previous
"""Causal self-attention TRN2 Bass kernel (B=4, T=2048, C=1024, H=16, D=64, fp32).

Sharding: 8 cores = 4 batches x 2 head-groups (8 heads each). Each core computes
its batch's QKV for its heads, causal flash-style attention, and a partial
output projection; the host sums the two head-group partials per batch.

v2: fully SBUF-resident, fused qc-outer pipeline.
  Per 512-query chunk tch (=qc):
    QKV: q_t[pr], k_sb[:, pr, tch], va[kb] computed from streamed x chunk
         (PE matmuls f32r; Pool drains PSUM->SBUF with bias add)
    attention qc=tch for all 4 head-pairs pr (needs only keys <= chunk end):
         S^T[k,q] strips (f32r, diagonal strips padded to >=256 rows),
         causal mask via DVE add of NEG triangle, exp on ACT (scale=1/8,
         bf16 out), AV with [V|ones] stationary (bf16) accumulating O^T and
         softmax sums in one PSUM tile; Pool drains, DVE reciprocal,
         DVE/Pool multiply -> oT (bf16)
    proj(tch): y^T partial = W_proj^T oT (bf16 x bf16), Pool bias-drain,
         DMA out.
Host: y[b] = (yT[2b] + yT[2b+1]).T
"""

import numpy as np
from contextlib import ExitStack

import concourse.bass as bass
import concourse.tile as tile
from concourse import bacc, mybir
from concourse.bass import ts
from concourse.bass_utils import run_bass_kernel_spmd

N_CORES = 8
B, T, C, H, D = 4, 2048, 1024, 16, 64
CB = C // 128          # 8 contraction blocks
NEG = -1.0e9

F32 = mybir.dt.float32
F32R = mybir.dt.float32r
BF16 = mybir.dt.bfloat16
AF = mybir.ActivationFunctionType
OP = mybir.AluOpType

_CACHE = {}

# query-strip low offset by diagonal position r (r = kb - 4*qc; r<0 off-diag)
_QLO = {0: 0, 1: 128, 2: 256, 3: 256}


def _build(phases=(1, 2, 3), reps=1):
    nc = bacc.Bacc("TRN2", target_bir_lowering=False, debug=False, num_devices=N_CORES)

    xT = nc.dram_tensor("xT", [C, T], F32R, kind="ExternalInput").ap()
    w_qk = nc.dram_tensor("w_qk", [C, 1024], F32R, kind="ExternalInput").ap()
    w_v = nc.dram_tensor("w_v", [C, 512], F32R, kind="ExternalInput").ap()
    w_pr = nc.dram_tensor("w_pr", [512, C], BF16, kind="ExternalInput").ap()
    b_qk = nc.dram_tensor("b_qk", [1024], F32, kind="ExternalInput").ap()
    b_v = nc.dram_tensor("b_v", [128, 512], F32, kind="ExternalInput").ap()
    b_pr = nc.dram_tensor("b_pr", [C], F32, kind="ExternalInput").ap()
    yT = nc.dram_tensor("yT", [C, T], F32, kind="ExternalOutput").ap()

    xT_r = xT.rearrange("(cb p) t -> p cb t", p=128)
    w_qk_r = w_qk.rearrange("(cb p) m -> p cb m", p=128)
    w_v_r = w_v.rearrange("(cb p) m -> p cb m", p=128)
    w_pr_r = w_pr.rearrange("(pb p) m -> p pb m", p=128)
    b_qk_r = b_qk.rearrange("(m p) -> p m", p=128)
    b_pr_r = b_pr.rearrange("(m p) -> p m", p=128)
    yT_r = yT.rearrange("(m p) t -> p m t", p=128)

    with tile.TileContext(nc) as tc:
        with ExitStack() as ctx:
            wqk_p = ctx.enter_context(tc.tile_pool(name="wqk", bufs=1))
            w2_p = ctx.enter_context(tc.tile_pool(name="w2", bufs=1))
            wpr_p = ctx.enter_context(tc.tile_pool(name="wpr", bufs=1))
            k_pl = ctx.enter_context(tc.tile_pool(name="kp", bufs=1))
            va_pl = ctx.enter_context(tc.tile_pool(name="vap", bufs=1))
            ot_pl = ctx.enter_context(tc.tile_pool(name="otp", bufs=1))
            x_pl = ctx.enter_context(tc.tile_pool(name="xp", bufs=2))
            q_pl = ctx.enter_context(tc.tile_pool(name="qp", bufs=2))
            p_pl = ctx.enter_context(tc.tile_pool(name="pp", bufs=4))
            ys_pl = ctx.enter_context(tc.tile_pool(name="ysp", bufs=2))
            rc_pl = ctx.enter_context(tc.tile_pool(name="rcp", bufs=1))
            misc = ctx.enter_context(tc.tile_pool(name="misc", bufs=1))
            ps_s = ctx.enter_context(tc.tile_pool(name="ps_s", bufs=3, space="PSUM"))
            ps_o = ctx.enter_context(tc.tile_pool(name="ps_o", bufs=1, space="PSUM"))

            # constants
            b_qk_sb = misc.tile([128, 8], F32)
            nc.sync.dma_start(b_qk_sb[:], b_qk_r)
            b_v_sb = misc.tile([128, 512], F32)
            nc.sync.dma_start(b_v_sb[:], b_v)
            b_pr_sb = misc.tile([128, 8], F32)
            nc.sync.dma_start(b_pr_sb[:], b_pr_r)
            # tri2: [128k, 256q] bf16 0/1: cols 0:128 all 0, cols 128:256
            # 1 where q>=k else 0 (q_rel = col-128, k = partition)
            tri2 = misc.tile([128, 256], BF16)
            nc.gpsimd.memset(tri2[:], 1.0)
            nc.gpsimd.affine_select(
                out=tri2[:], in_=tri2[:], compare_op=OP.is_ge, fill=0.0,
                base=-128, pattern=[[1, 256]], channel_multiplier=-1,
            )
            tri = tri2[:, 128:256]

            # weights
            w_qk_sb = wqk_p.tile([128, CB, 1024], F32R)
            nc.sync.dma_start(w_qk_sb[:], w_qk_r)
            w_v_sb = w2_p.tile([128, CB, 512], F32R)
            nc.sync.dma_start(w_v_sb[:], w_v_r)
            w_pr_sb = wpr_p.tile([128, 4, 1024], BF16)
            nc.sync.dma_start(w_pr_sb[:], w_pr_r)

            # persistent activations (shared across reps; rewritten per rep)
            k_sb = k_pl.tile([128, 4, T], F32R, name="k_sb")
            va = va_pl.tile([128, 16, 8, 2, 64], BF16, name="va")
            oT = ot_pl.tile([128, 4, T], BF16, name="oT")
            nc.gpsimd.memset(va[:, :, :, 1, :], 1.0)

            for _rep in range(reps):
                for tch in range(4):
                    # ---- QKV for this 512-t chunk ----
                    x_t = x_pl.tile([128, CB, 512], F32R, tag="x",
                                    name=f"x_{_rep}_{tch}")
                    nc.sync.dma_start(x_t[:], xT_r[:, :, ts(tch, 512)])
                    q_t = q_pl.tile([128, 4, 512], F32R, tag="q",
                                    name=f"q_{_rep}_{tch}")
                    for mp in range(4):
                        ps = ps_s.tile([128, 2, 512], F32, tag="ps_s",
                                       name=f"qk_{_rep}_{tch}_{mp}")
                        for cb in range(CB):
                            for h in (0, 1):
                                nc.tensor.matmul(
                                    ps[:, h], w_qk_sb[:, cb, ts(2 * mp + h, 128)],
                                    x_t[:, cb],
                                    start=(cb == 0), stop=(cb == CB - 1),
                                )
                        for h in (0, 1):
                            m = 2 * mp + h
                            if m < 4:
                                nc.vector.tensor_scalar_add(
                                    q_t[:, m, :], ps[:, h], b_qk_sb[:, m : m + 1])
                            else:
                                nc.vector.tensor_scalar_add(
                                    k_sb[:, m - 4, ts(tch, 512)], ps[:, h],
                                    b_qk_sb[:, m : m + 1])
                    for vp in range(2):
                        ps = ps_s.tile([128, 2, 512], F32, tag="ps_s",
                                       name=f"v_{_rep}_{tch}_{vp}")
                        for cb in range(CB):
                            for h in (0, 1):
                                nc.tensor.matmul(
                                    ps[:, h], x_t[:, cb, ts(2 * vp + h, 128)],
                                    w_v_sb[:, cb],
                                    start=(cb == 0), stop=(cb == CB - 1),
                                )
                        for h in (0, 1):
                            tb = tch * 4 + 2 * vp + h
                            nc.vector.tensor_tensor(
                                va[:, tb, :, 0, :],
                                ps[:, h].rearrange("p (s d) -> p s d", s=8),
                                b_v_sb.rearrange("p (s d) -> p s d", s=8),
                                OP.add)

                    # ---- attention qc = tch, all head-pairs ----
                    qc = tch
                    nkb = 4 * qc + 4
                    for pr in range(4 if 2 in phases else 0):
                        po = ps_o.tile([128, 2, 512], F32, tag="ps_o",
                                       name=f"o_{_rep}_{qc}_{pr}")
                        s_tiles = {}

                        def s_step(kb, pr=pr, s_tiles=s_tiles):
                            r = kb - 4 * qc
                            qlo = _QLO.get(r, 0)
                            s = ps_s.tile([128, 2, 512], F32, tag="ps_s",
                                          name=f"s_{_rep}_{qc}_{pr}_{kb}")
                            for j in (0, 1):
                                pb = 64 * j
                                nc.tensor.matmul(
                                    s[:, j, qlo:512],
                                    k_sb[pb : pb + 64, pr, ts(kb, 128)],
                                    q_t[pb : pb + 64, pr, qlo:512],
                                    start=True, stop=True, tile_position=(pb, 0),
                                )
                            s_tiles[kb] = (s, qlo)

                        def av_step(kb, pr=pr, po=po, s_tiles=s_tiles):
                            s, qlo = s_tiles.pop(kb)
                            r = kb - 4 * qc
                            p_t = p_pl.tile([128, 2, 512], BF16, tag="p",
                                            name=f"p_{_rep}_{qc}_{pr}_{kb}")
                            nc.scalar.activation(
                                p_t[:, :, qlo:512], s[:, :, qlo:512],
                                AF.Exp, scale=0.125)
                            if 0 <= r <= 2:
                                c0 = 128 * r
                                nc.gpsimd.tensor_tensor(
                                    p_t[:, :, c0 : c0 + 128], p_t[:, :, c0 : c0 + 128],
                                    tri[:, None, :].to_broadcast((128, 2, 128)),
                                    OP.mult)
                            elif r == 3:
                                nc.gpsimd.tensor_tensor(
                                    p_t[:, :, 256:512], p_t[:, :, 256:512],
                                    tri2[:, None, :].to_broadcast((128, 2, 256)),
                                    OP.mult)
                            for j in (0, 1):
                                s0 = 2 * pr + j
                                nc.tensor.matmul(
                                    po[:, j, qlo:512],
                                    va[:, kb, s0].rearrange("p a b -> p (a b)"),
                                    p_t[:, j, qlo:512],
                                    start=(kb == 0), stop=(kb == nkb - 1),
                                )

                        s_step(0)
                        if nkb > 1:
                            s_step(1)
                        for kb in range(nkb):
                            if kb + 2 < nkb:
                                s_step(kb + 2)
                            av_step(kb)

                        # normalize straight from PSUM
                        rec = rc_pl.tile([64, 2, 512], F32, tag="rec",
                                         name=f"rec_{_rep}_{qc}_{pr}")
                        nc.vector.reciprocal(rec[:], po[64:128])
                        for j in (0, 1):
                            nc.vector.tensor_tensor(
                                oT[j * 64 : (j + 1) * 64, pr, ts(qc, 512)],
                                po[0:64, j], rec[:, j], OP.mult)

                    # ---- projection for this chunk ----
                    for mp in range(4 if 3 in phases else 0):
                        ps = ps_s.tile([128, 2, 512], F32, tag="ps_s",
                                       name=f"y_{_rep}_{tch}_{mp}")
                        for h in (0, 1):
                            for pb in range(4):
                                nc.tensor.matmul(
                                    ps[:, h], w_pr_sb[:, pb, ts(2 * mp + h, 128)],
                                    oT[:, pb, ts(tch, 512)],
                                    start=(pb == 0), stop=(pb == 3),
                                )
                        yst = ys_pl.tile([128, 2, 512], F32, tag="ystg",
                                         name=f"yst_{_rep}_{tch}_{mp}")
                        for h in (0, 1):
                            nc.vector.tensor_scalar_add(
                                yst[:, h], ps[:, h],
                                b_pr_sb[:, 2 * mp + h : 2 * mp + h + 1])
                        nc.sync.dma_start(
                            yT_r[:, 2 * mp : 2 * mp + 2, ts(tch, 512)], yst[:])

    nc.compile()
    return nc


def _in_maps(x, W_attn, b_attn, W_proj, b_proj):
    maps = []
    for b in range(B):
        for g in range(2):
            cs = slice(g * 512, (g + 1) * 512)
            maps.append({
                "xT": np.ascontiguousarray(x[b].T),
                "w_qk": np.ascontiguousarray(
                    np.concatenate([W_attn[:, cs], W_attn[:, 1024 + cs.start : 1024 + cs.stop]], axis=1)),
                "w_v": np.ascontiguousarray(W_attn[:, 2048 + cs.start : 2048 + cs.stop]),
                "w_pr": np.ascontiguousarray(W_proj[cs, :]).astype(mybir.dt.np(BF16)),
                "b_qk": np.ascontiguousarray(
                    np.concatenate([b_attn[cs], b_attn[1024 + cs.start : 1024 + cs.stop]])),
                "b_v": np.ascontiguousarray(
                    np.tile(b_attn[2048 + cs.start : 2048 + cs.stop][None, :], (128, 1))),
                "b_pr": np.ascontiguousarray(b_proj),
            })
    return maps


def kernel(x, W_attn, b_attn, W_proj, b_proj):
    x = np.asarray(x, dtype=np.float32)
    W_attn = np.asarray(W_attn, dtype=np.float32)
    b_attn = np.asarray(b_attn, dtype=np.float32)
    W_proj = np.asarray(W_proj, dtype=np.float32)
    b_proj = np.asarray(b_proj, dtype=np.float32)

    if "nc" not in _CACHE:
        _CACHE["nc"] = _build()
    nc = _CACHE["nc"]

    maps = _in_maps(x, W_attn, b_attn, W_proj, b_proj)
    last_exc = None
    for attempt in range(3):
        try:
            res = run_bass_kernel_spmd(nc, maps, core_ids=list(range(N_CORES)))
            break
        except Exception as exc:  # transient device wedges recover on retry
            last_exc = exc
            if attempt == 2:
                raise
            import time as _time
            _time.sleep(5)
    y = np.empty((B, T, C), dtype=np.float32)
    for b in range(B):
        y[b] = (res.results[2 * b]["yT"] + res.results[2 * b + 1]["yT"]).T
    return y
